# revision 2
# baseline (speedup 1.0000x reference)
"""Trainium2 Bass kernel for batched multi-head self-attention block — v2.

Single-pass fp16 design (rel-err budget 2e-2 allows it; measured ~5.5e-4):
  - host pre-transposes x and converts all operands to fp16
  - qkv projection, scores, ctx, out-projection all single fp16 matmuls
  - exp on Act engine reading [128,1024] psum tiles
  - softmax normalization via DRAM-roundtrip partition-broadcast of 1/denom

Sharding: 8 cores = 2 batches x 4 head-groups (4 heads each). Host sums the
4 head-group partials per batch and adds bout.
"""

import numpy as np
import os
DEBUG_DUMP = bool(os.environ.get("KDBG"))

B, T, D, H, HD = 2, 2048, 1024, 16, 64
NCORES = 8
NHEADS = 4            # heads per core
NQK = NHEADS * HD     # 256
DT = D // 128         # 8 d-tiles
TT = T // 128         # 16 t-tiles
QB = 1024             # q-block
NQB = T // QB         # 2
KT = T // 128         # 16 k-tiles


def _patch_tile_drain():
    """walrus CoreV3 rejects >2 sem waits on one CTRL instruction; split the
    Tile kernel-tail drain waits across single-wait nops."""
    import concourse.tile as tile
    import concourse.mybir as mybir
    from concourse.vector_clock import ScopedClock

    if getattr(tile.TileContext, "_drain_patched", False):
        return

    def _drain_and_barrier_split(self, tick_clock, wait_clock):
        nc = self.nc
        drain_inst = nc.sync.drain()
        wait_clock.add_sem_waits(
            drain_inst.ins, ScopedClock({None: tick_clock.global_clock})
        )
        mi = drain_inst.ins
        si = getattr(mi, "sync_info", None)
        waits = list(si.on_wait or []) if si is not None else []
        if len(waits) > 1:
            si.on_wait = waits[:1]
            for w in waits[1:]:
                nop = nc.sync.nop().ins
                if getattr(nop, "sync_info", None) is None:
                    nop.sync_info = mybir.SyncInfo(on_wait=[w], on_update=[])
                else:
                    nop.sync_info.on_wait = [w]

        nc.all_engine_barrier()
        assert self.sems is not None
        popped = nc._tile_sem_poison_stack.pop()
        assert popped is self._sem_poison
        nc.clear_and_free_semaphores(list(self.sems.allocated().values()))
        nc.all_engine_barrier()

    tile.TileContext._drain_and_barrier = _drain_and_barrier_split
    tile.TileContext._drain_patched = True


def split_excess_waits(nc, max_waits=1):
    """walrus CoreV3 in this env accepts at most 1 sync-wait per instruction;
    move extras onto same-engine nops inserted just before."""
    import concourse.mybir as mybir

    ctr = 0
    for f in nc.m.functions:
        for b in f.blocks:
            newlist = []
            changed = False
            for inst in b.instructions:
                si = getattr(inst, "sync_info", None)
                waits = list(si.on_wait or []) if si is not None else []
                if len(waits) > max_waits:
                    assert inst.engine != mybir.EngineType.Unassigned, inst
                    for w in waits[:-max_waits]:
                        ctr += 1
                        nop = mybir.InstNoOp(name=f"waitnop-{ctr}", ins=[], outs=[])
                        nop.engine = inst.engine
                        nop.sync_info = mybir.SyncInfo(on_wait=[w], on_update=[])
                        newlist.append(nop)
                    si.on_wait = waits[-max_waits:]
                    changed = True
                newlist.append(inst)
            if changed:
                b.instructions = newlist
    return ctr


def build_nc(loop_n=None):
    import concourse.bass as bass
    import concourse.mybir as mybir
    import concourse.tile as tile
    from contextlib import ExitStack

    _patch_tile_drain()
    f32 = mybir.dt.float32
    f16 = mybir.dt.float16
    EXP = mybir.ActivationFunctionType.Exp
    ADD = mybir.AluOpType.add

    from concourse.tile_rust import add_dep_helper

    def chain(mms):
        for a, b_ in zip(mms[1:], mms[:-1]):
            add_dep_helper(a.ins, b_.ins, sync=False, reason="psum group order")

    nc = bass.Bass()
    xtd = nc.declare_dram_parameter("xt16", [D, T], f16, isOutput=False)
    wqkd = nc.declare_dram_parameter("wqk16", [D, 2 * NQK], f16, isOutput=False)
    wvd = nc.declare_dram_parameter("wv16", [D, NQK], f16, isOutput=False)
    woutd = nc.declare_dram_parameter("wout16", [NQK, D], f16, isOutput=False)
    bqkd = nc.declare_dram_parameter("bqk", [128, 4], f32, isOutput=False)
    bvrepd = nc.declare_dram_parameter("bvrep", [1, NHEADS * 16 * HD], f16, isOutput=False)
    out = nc.declare_dram_parameter("out", [T, D], f32, isOutput=True)
    dbg = {}
    if DEBUG_DUMP:
        dbg["qk"] = nc.declare_dram_parameter("dbg_qk", [128, 4 * T], f16, isOutput=True)
        dbg["vaug"] = nc.declare_dram_parameter("dbg_vaug", [128, TT * NHEADS * (HD + 1)], f16, isOutput=True)
        dbg["at0"] = nc.declare_dram_parameter("dbg_at0", [128, KT * QB], f16, isOutput=True)
        dbg["cxt"] = nc.declare_dram_parameter("dbg_cxt", [128, 2 * T], f16, isOutput=True)
        dbg["scol"] = nc.declare_dram_parameter("dbg_scol", [1, NHEADS * QB], f32, isOutput=True)

    screc = nc.dram_tensor("screc", [2 * NQB * NHEADS, QB], f32)

    with tile.TileContext(nc) as tc, ExitStack() as ctx:
        const_p = ctx.enter_context(tc.tile_pool(name="const", bufs=1))
        big_p = ctx.enter_context(tc.tile_pool(name="big", bufs=1))

        ones_sb = const_p.tile([1, 128], f16, tag="ones")
        nc.vector.memset(ones_sb, 1.0)

        wqk_sb = const_p.tile([128, DT, 2 * NQK], f16, tag="wqk")
        wv_sb = const_p.tile([128, DT, NQK], f16, tag="wv")
        wout_sb = const_p.tile([128, 2, D], f16, tag="wout")
        nc.sync.dma_start(out=wqk_sb, in_=wqkd.rearrange("(dt p) n -> p dt n", p=128))
        nc.sync.dma_start(out=wv_sb, in_=wvd.rearrange("(dt p) n -> p dt n", p=128))
        nc.sync.dma_start(out=wout_sb, in_=woutd.rearrange("(kt p) n -> p kt n", p=128))
        bqk_sb = const_p.tile([128, 4], f32, tag="bqk")
        nc.sync.dma_start(out=bqk_sb, in_=bqkd[:, :])
        bvrep_sb = const_p.tile([1, NHEADS * 16 * HD], f16, tag="bvrep")
        nc.sync.dma_start(out=bvrep_sb, in_=bvrepd[:, :])

        # persistent activations
        xt_sb = big_p.tile([128, DT, T], f16, tag="xt")
        qk_sb = big_p.tile([128, 4, T], f16, tag="qk")       # m: q01,q23,k01,k23
        vaug = big_p.tile([128, TT, NHEADS * (HD + 1)], f16, tag="vaug")
        cxt = big_p.tile([128, 2, T], f16, tag="cxt")        # ctxT, normalized in place
        at_sb = [
            big_p.tile([128, KT, QB], f16, tag=f"at{i}", name=f"at{i}")
            for i in range(2)
        ]
        scol = big_p.tile([1, NHEADS * QB], f32, tag="scol")
        rec = big_p.tile([1, NHEADS * QB], f32, tag="rec")
        rb = big_p.tile([128, QB], f32, tag="rb")

        # ones columns of v_aug (never overwritten by v copies)
        nc.vector.memset(
            vaug.rearrange("p t (h c) -> p t h c", h=NHEADS)[:, :, :, HD : HD + 1],
            1.0,
        )

        loop_cm = tc.For_i(0, loop_n, 1) if loop_n else None
        if loop_cm is not None:
            loop_cm.__enter__()

        # ---- load xT ----
        for dt in range(DT):
            nc.sync.dma_start(
                out=xt_sb[:, dt, :],
                in_=xtd.rearrange("(dt p) t -> p dt t", p=128)[:, dt, :],
            )

        def qk_tile(qp, m, th):
            ps = qp.tile([128, QB], f32, tag="qkps")
            mms = []
            for s in range(2):
                for dt in range(DT):
                    mms.append(nc.tensor.matmul(
                        ps[:, s * 512 : (s + 1) * 512],
                        lhsT=wqk_sb[:, dt, m * 128 : (m + 1) * 128],
                        rhs=xt_sb[:, dt, th * QB + s * 512 : th * QB + (s + 1) * 512],
                        start=(dt == 0),
                        stop=(dt == DT - 1),
                        skip_group_check=True,
                    ))
            chain(mms)
            nc.vector.tensor_scalar_add(
                out=qk_sb[:, m, th * QB : (th + 1) * QB],
                in0=ps,
                scalar1=bqk_sb[:, m : m + 1],
            )

        def scores_exp(sp, h, qblk, buf):
            """16 k-tiles of scores + exp for (h, q-block)."""
            qrow = 64 * (h % 2)
            at = at_sb[buf]
            for kt in range(KT):
                sps = sp.tile([128, QB], f32, tag="sps")
                for s in range(2):
                    nc.tensor.matmul(
                        sps[:, s * 512 : (s + 1) * 512],
                        lhsT=qk_sb[qrow : qrow + 64, 2 + h // 2, kt * 128 : (kt + 1) * 128],
                        rhs=qk_sb[qrow : qrow + 64, h // 2,
                                  qblk * QB + s * 512 : qblk * QB + (s + 1) * 512],
                        start=True, stop=True, skip_group_check=True,
                    )
                nc.scalar.activation(at[:, kt, :], sps, EXP, scale=0.125)

        def ctx_head(cp, h, qblk, buf):
            at = at_sb[buf]
            cps = cp.tile([HD + 1, QB], f32, tag="cps")
            mms = []
            for kt in range(KT):
                for s in range(2):
                    mms.append(nc.tensor.matmul(
                        cps[:, s * 512 : (s + 1) * 512],
                        lhsT=vaug[:, kt, h * (HD + 1) : (h + 1) * (HD + 1)],
                        rhs=at[:, kt, s * 512 : (s + 1) * 512],
                        start=(kt == 0), stop=(kt == KT - 1),
                        skip_group_check=True,
                    ))
            # order the two interleaved accumulation groups
            chain(mms[0::2])
            chain(mms[1::2])
            nc.vector.tensor_copy(
                out=cxt[64 * (h % 2) : 64 * (h % 2) + 64, h // 2,
                        qblk * QB : (qblk + 1) * QB],
                in_=cps[0:HD, :],
            )
            nc.vector.tensor_copy(
                out=scol[0:1, h * QB : (h + 1) * QB], in_=cps[HD : HD + 1, :]
            )

        def normalize(qblk):
            nc.vector.reciprocal(rec, scol)
            nc.sync.dma_start(
                out=screc.rearrange("r q -> (r q)")[
                    qblk * NHEADS * QB : (qblk + 1) * NHEADS * QB
                ],
                in_=rec,
            )
            for kt in range(2):
                bsrc = bass.AP(
                    tensor=screc[:].tensor,
                    offset=(qblk * NHEADS + kt * 2) * QB,
                    ap=[[QB, 2], [0, 64], [1, QB]],
                )
                nc.sync.dma_start(out=rb, in_=bsrc)
                nc.vector.tensor_mul(
                    cxt[:, kt, qblk * QB : (qblk + 1) * QB],
                    cxt[:, kt, qblk * QB : (qblk + 1) * QB],
                    rb,
                )

        sp_cm = tc.tile_pool(name="sps", bufs=3, space="PSUM")
        sp = sp_cm.__enter__()
        qk_cm = tc.tile_pool(name="qkps", bufs=1, space="PSUM")
        qp = qk_cm.__enter__()

        # qk tiles needed by scores(h=0, qblk=0): q01 t-half0, k01 both halves
        qk_tile(qp, 0, 0)
        qk_tile(qp, 2, 0)
        qk_tile(qp, 2, 1)
        scores_exp(sp, 0, 0, 0)
        qk_tile(qp, 0, 1)
        qk_tile(qp, 1, 0)
        qk_tile(qp, 1, 1)
        qk_tile(qp, 3, 0)
        qk_tile(qp, 3, 1)
        qk_cm.__exit__(None, None, None)

        # ---- v projection (natural [t, n]) ----
        v_cm = tc.tile_pool(name="vps", bufs=1, space="PSUM")
        vp = v_cm.__enter__()
        for h in range(NHEADS):
            vps = vp.tile([128, TT * HD], f32, tag="vps")
            mms = []
            for tt in range(TT):
                col = tt * HD
                # bias first so each tt accumulation group is contiguous:
                # PSUM allows only one open group per bank at a time
                mms.append(nc.tensor.matmul(
                    vps[:, col : col + HD],
                    lhsT=ones_sb,
                    rhs=bvrep_sb[0:1, h * 1024 + col : h * 1024 + col + HD],
                    start=True, stop=False, skip_group_check=True,
                ))
                for dt in range(DT):
                    mms.append(nc.tensor.matmul(
                        vps[:, col : col + HD],
                        lhsT=xt_sb[:, dt, tt * 128 : (tt + 1) * 128],
                        rhs=wv_sb[:, dt, h * HD : (h + 1) * HD],
                        start=False, stop=(dt == DT - 1), skip_group_check=True,
                    ))
            chain(mms)
            nc.vector.tensor_copy(
                out=vaug.rearrange("p t (h c) -> p t h c", h=NHEADS)[:, :, h, 0:HD],
                in_=vps.rearrange("p (t c) -> p t c", c=HD),
            )
        v_cm.__exit__(None, None, None)

        # ---- attention (q-block major) ----
        cp_cm = tc.tile_pool(name="cps", bufs=1, space="PSUM")
        cp = cp_cm.__enter__()
        ctx_head(cp, 0, 0, 0)
        for qblk in range(NQB):
            for h in range(NHEADS):
                if qblk == 0 and h == 0:
                    continue
                buf = (qblk * NHEADS + h) % 2
                scores_exp(sp, h, qblk, buf)
                ctx_head(cp, h, qblk, buf)
            normalize(qblk)
        cp_cm.__exit__(None, None, None)
        sp_cm.__exit__(None, None, None)

        if DEBUG_DUMP:
            nc.sync.dma_start(out=dbg["qk"][:, :], in_=qk_sb.rearrange("p m t -> p (m t)"))
            nc.sync.dma_start(out=dbg["vaug"][:, :], in_=vaug.rearrange("p t c -> p (t c)"))
            nc.sync.dma_start(out=dbg["at0"][:, :], in_=at_sb[0].rearrange("p k q -> p (k q)"))
            nc.sync.dma_start(out=dbg["cxt"][:, :], in_=cxt.rearrange("p k t -> p (k t)"))
            nc.sync.dma_start(out=dbg["scol"][:, :], in_=scol)

        # ---- out projection ----
        with (
            tc.tile_pool(name="ops", bufs=2, space="PSUM") as o_p,
            tc.tile_pool(name="ostage", bufs=3) as out_p,
        ):
            for tt in range(TT):
                for nb in range(2):
                    ops = o_p.tile([128, 512], f32, tag="ops")
                    mms = []
                    for kt in range(2):
                        mms.append(nc.tensor.matmul(
                            ops,
                            lhsT=cxt[:, kt, tt * 128 : (tt + 1) * 128],
                            rhs=wout_sb[:, kt, nb * 512 : (nb + 1) * 512],
                            start=(kt == 0), stop=(kt == 1),
                            skip_group_check=True,
                        ))
                    chain(mms)
                    ot = out_p.tile([128, 512], f32, tag="ot")
                    nc.vector.tensor_copy(ot, ops)
                    nc.sync.dma_start(
                        out=out[tt * 128 : (tt + 1) * 128, nb * 512 : (nb + 1) * 512],
                        in_=ot,
                    )

        if loop_cm is not None:
            loop_cm.__exit__(None, None, None)

    return nc


_NC_CACHE = None


def _get_nc():
    global _NC_CACHE
    if _NC_CACHE is None:
        nc = build_nc()
        split_excess_waits(nc)
        _NC_CACHE = nc
    return _NC_CACHE


def make_in_maps(x, Wqkv, bqkv, Wout):
    x = np.asarray(x, dtype=np.float32)
    Wqkv = np.asarray(Wqkv, dtype=np.float32)
    bqkv = np.asarray(bqkv, dtype=np.float32)
    Wout = np.asarray(Wout, dtype=np.float32)
    in_maps = []
    for c in range(NCORES):
        b, g = divmod(c, 4)
        qs = slice(NQK * g, NQK * (g + 1))
        ks = slice(D + NQK * g, D + NQK * (g + 1))
        vs = slice(2 * D + NQK * g, 2 * D + NQK * (g + 1))
        qb, kb, vb = bqkv[qs], bqkv[ks], bqkv[vs]
        bqk_host = np.stack(
            [qb[0:128], qb[128:256], kb[0:128], kb[128:256]], axis=1
        )
        bvrep = np.ascontiguousarray(
            np.broadcast_to(vb.reshape(NHEADS, 1, HD), (NHEADS, 16, HD))
        ).reshape(1, NHEADS * 16 * HD)
        in_maps.append(
            {
                "xt16": np.ascontiguousarray(x[b].T.astype(np.float16)),
                "wqk16": np.ascontiguousarray(
                    np.concatenate([Wqkv[:, qs], Wqkv[:, ks]], axis=1)
                ).astype(np.float16),
                "wv16": np.ascontiguousarray(Wqkv[:, vs]).astype(np.float16),
                "wout16": np.ascontiguousarray(
                    Wout[NQK * g : NQK * (g + 1), :]
                ).astype(np.float16),
                "bqk": np.ascontiguousarray(bqk_host),
                "bvrep": bvrep.astype(np.float16),
            }
        )
    return in_maps


def gather_out(results, bout):
    bout = np.asarray(bout, dtype=np.float32)
    outs = [np.asarray(results[c]["out"], dtype=np.float32) for c in range(NCORES)]
    full = np.stack(
        [outs[4 * b] + outs[4 * b + 1] + outs[4 * b + 2] + outs[4 * b + 3]
         for b in range(B)]
    )
    return (full + bout[None, None, :]).astype(np.float32)


def kernel(x, Wqkv, bqkv, Wout, bout):
    from concourse.bass_utils import run_bass_kernel_spmd

    nc = _get_nc()
    in_maps = make_in_maps(x, Wqkv, bqkv, Wout)
    res = run_bass_kernel_spmd(nc, in_maps, list(range(NCORES)))
    return gather_out(res.results, bout)


# revision 3
# speedup vs baseline: 1.0334x; 1.0334x over previous
"""Trainium2 Bass kernel for batched multi-head self-attention block — v2.

Single-pass fp16 design (rel-err budget 2e-2 allows it; measured ~5.5e-4):
  - host pre-transposes x and converts all operands to fp16
  - qkv projection, scores, ctx, out-projection all single fp16 matmuls
  - exp on Act engine reading [128,1024] psum tiles
  - softmax normalization via DRAM-roundtrip partition-broadcast of 1/denom

Sharding: 8 cores = 2 batches x 4 head-groups (4 heads each). Host sums the
4 head-group partials per batch and adds bout.
"""

import numpy as np
import os
DEBUG_DUMP = bool(os.environ.get("KDBG"))

B, T, D, H, HD = 2, 2048, 1024, 16, 64
NCORES = 8
NHEADS = 4            # heads per core
NQK = NHEADS * HD     # 256
DT = D // 128         # 8 d-tiles
TT = T // 128         # 16 t-tiles
QB = 1024             # q-block
NQB = T // QB         # 2
KT = T // 128         # 16 k-tiles


def _patch_tile_drain():
    """walrus CoreV3 rejects >2 sem waits on one CTRL instruction; split the
    Tile kernel-tail drain waits across single-wait nops."""
    import concourse.tile as tile
    import concourse.mybir as mybir
    from concourse.vector_clock import ScopedClock

    if getattr(tile.TileContext, "_drain_patched", False):
        return

    def _drain_and_barrier_split(self, tick_clock, wait_clock):
        nc = self.nc
        drain_inst = nc.sync.drain()
        wait_clock.add_sem_waits(
            drain_inst.ins, ScopedClock({None: tick_clock.global_clock})
        )
        mi = drain_inst.ins
        si = getattr(mi, "sync_info", None)
        waits = list(si.on_wait or []) if si is not None else []
        if len(waits) > 1:
            si.on_wait = waits[:1]
            for w in waits[1:]:
                nop = nc.sync.nop().ins
                if getattr(nop, "sync_info", None) is None:
                    nop.sync_info = mybir.SyncInfo(on_wait=[w], on_update=[])
                else:
                    nop.sync_info.on_wait = [w]

        nc.all_engine_barrier()
        assert self.sems is not None
        popped = nc._tile_sem_poison_stack.pop()
        assert popped is self._sem_poison
        nc.clear_and_free_semaphores(list(self.sems.allocated().values()))
        nc.all_engine_barrier()

    tile.TileContext._drain_and_barrier = _drain_and_barrier_split
    tile.TileContext._drain_patched = True


def split_excess_waits(nc, max_waits=1):
    """walrus CoreV3 in this env accepts at most 1 sync-wait per instruction;
    move extras onto same-engine nops inserted just before."""
    import concourse.mybir as mybir

    ctr = 0
    for f in nc.m.functions:
        for b in f.blocks:
            newlist = []
            changed = False
            for inst in b.instructions:
                si = getattr(inst, "sync_info", None)
                waits = list(si.on_wait or []) if si is not None else []
                if len(waits) > max_waits:
                    assert inst.engine != mybir.EngineType.Unassigned, inst
                    for w in waits[:-max_waits]:
                        ctr += 1
                        nop = mybir.InstNoOp(name=f"waitnop-{ctr}", ins=[], outs=[])
                        nop.engine = inst.engine
                        nop.sync_info = mybir.SyncInfo(on_wait=[w], on_update=[])
                        newlist.append(nop)
                    si.on_wait = waits[-max_waits:]
                    changed = True
                newlist.append(inst)
            if changed:
                b.instructions = newlist
    return ctr


def build_nc(loop_n=None):
    import concourse.bass as bass
    import concourse.mybir as mybir
    import concourse.tile as tile
    from contextlib import ExitStack

    _patch_tile_drain()
    f32 = mybir.dt.float32
    f16 = mybir.dt.float16
    EXP = mybir.ActivationFunctionType.Exp
    ADD = mybir.AluOpType.add

    from concourse.tile_rust import add_dep_helper

    def chain(mms):
        for a, b_ in zip(mms[1:], mms[:-1]):
            add_dep_helper(a.ins, b_.ins, sync=False, reason="psum group order")

    nc = bass.Bass()
    xtd = nc.declare_dram_parameter("xt16", [D, T], f16, isOutput=False)
    wqkd = nc.declare_dram_parameter("wqk16", [D, 2 * NQK], f16, isOutput=False)
    wvd = nc.declare_dram_parameter("wv16", [D, NQK], f16, isOutput=False)
    woutd = nc.declare_dram_parameter("wout16", [NQK, D], f16, isOutput=False)
    bqkd = nc.declare_dram_parameter("bqk", [128, 4], f32, isOutput=False)
    bvrepd = nc.declare_dram_parameter("bvrep", [1, NHEADS * 16 * HD], f16, isOutput=False)
    out = nc.declare_dram_parameter("out", [T, D], f32, isOutput=True)
    dbg = {}
    if DEBUG_DUMP:
        dbg["qk"] = nc.declare_dram_parameter("dbg_qk", [128, 4 * T], f16, isOutput=True)
        dbg["vaug"] = nc.declare_dram_parameter("dbg_vaug", [128, TT * NHEADS * (HD + 1)], f16, isOutput=True)
        dbg["at0"] = nc.declare_dram_parameter("dbg_at0", [128, KT * QB], f16, isOutput=True)
        dbg["cxt"] = nc.declare_dram_parameter("dbg_cxt", [128, 2 * T], f16, isOutput=True)
        dbg["scol"] = nc.declare_dram_parameter("dbg_scol", [1, NHEADS * QB], f32, isOutput=True)

    screc = nc.dram_tensor("screc", [2 * NQB * NHEADS, QB], f32)

    with tile.TileContext(nc) as tc, ExitStack() as ctx:
        const_p = ctx.enter_context(tc.tile_pool(name="const", bufs=1))
        big_p = ctx.enter_context(tc.tile_pool(name="big", bufs=1))

        ones_sb = const_p.tile([1, 128], f16, tag="ones")
        nc.vector.memset(ones_sb, 1.0)

        wqk_sb = const_p.tile([128, DT, 2 * NQK], f16, tag="wqk")
        wv_sb = const_p.tile([128, DT, NQK], f16, tag="wv")
        wout_sb = const_p.tile([128, 2, D], f16, tag="wout")
        nc.sync.dma_start(out=wqk_sb, in_=wqkd.rearrange("(dt p) n -> p dt n", p=128))
        nc.sync.dma_start(out=wv_sb, in_=wvd.rearrange("(dt p) n -> p dt n", p=128))
        nc.sync.dma_start(out=wout_sb, in_=woutd.rearrange("(kt p) n -> p kt n", p=128))
        bqk_sb = const_p.tile([128, 4], f32, tag="bqk")
        nc.sync.dma_start(out=bqk_sb, in_=bqkd[:, :])
        bvrep_sb = const_p.tile([1, NHEADS * 16 * HD], f16, tag="bvrep")
        nc.sync.dma_start(out=bvrep_sb, in_=bvrepd[:, :])

        # persistent activations
        xt_sb = big_p.tile([128, DT, T], f16, tag="xt")
        qk_sb = big_p.tile([128, 4, T], f16, tag="qk")       # m: q01,q23,k01,k23
        vaug = big_p.tile([128, TT, NHEADS * (HD + 1)], f16, tag="vaug")
        cxt = big_p.tile([128, 2, T], f16, tag="cxt")        # ctxT, normalized in place
        at_sb = [
            big_p.tile([128, KT, QB], f16, tag=f"at{i}", name=f"at{i}")
            for i in range(2)
        ]
        scol = big_p.tile([1, NHEADS * QB], f32, tag="scol")
        rec = big_p.tile([1, NHEADS * QB], f32, tag="rec")
        rb = big_p.tile([128, QB], f32, tag="rb")

        # ones columns of v_aug (never overwritten by v copies)
        nc.vector.memset(
            vaug.rearrange("p t (h c) -> p t h c", h=NHEADS)[:, :, :, HD : HD + 1],
            1.0,
        )

        # preload Exp activation table (outside the timed loop)
        tscr = const_p.tile([1, 1], f32, tag="tscr")
        nc.scalar.activation(tscr, bqk_sb[0:1, 0:1], EXP)

        loop_cm = tc.For_i(0, loop_n, 1) if loop_n else None
        if loop_cm is not None:
            loop_cm.__enter__()

        # ---- load xT ----
        for dt in range(DT):
            nc.sync.dma_start(
                out=xt_sb[:, dt, :],
                in_=xtd.rearrange("(dt p) t -> p dt t", p=128)[:, dt, :],
            )

        def qk_tile(qp, m, th):
            ps = qp.tile([128, QB], f32, tag="qkps")
            mms = []
            for s in range(2):
                for dt in range(DT):
                    mms.append(nc.tensor.matmul(
                        ps[:, s * 512 : (s + 1) * 512],
                        lhsT=wqk_sb[:, dt, m * 128 : (m + 1) * 128],
                        rhs=xt_sb[:, dt, th * QB + s * 512 : th * QB + (s + 1) * 512],
                        start=(dt == 0),
                        stop=(dt == DT - 1),
                        skip_group_check=True,
                    ))
            chain(mms)
            nc.vector.tensor_scalar_add(
                out=qk_sb[:, m, th * QB : (th + 1) * QB],
                in0=ps,
                scalar1=bqk_sb[:, m : m + 1],
            )

        def _score_mm(h, qblk, kt, dst):
            qrow = 64 * (h % 2)
            for s in range(2):
                nc.tensor.matmul(
                    dst[:, s * 512 : (s + 1) * 512],
                    lhsT=qk_sb[qrow : qrow + 64, 2 + h // 2, kt * 128 : (kt + 1) * 128],
                    rhs=qk_sb[qrow : qrow + 64, h // 2,
                              qblk * QB + s * 512 : qblk * QB + (s + 1) * 512],
                    start=True, stop=True, skip_group_check=True,
                )

        def scores_exp(sp, h, qblk, buf):
            """scores+exp with mixed 2048/1024-wide exp instrs (A/B ring)."""
            at = at_sb[buf]
            for tr in range(5):
                k0 = 3 * tr
                _score_mm(h, qblk, k0, spsA[:, 0, :])
                _score_mm(h, qblk, k0 + 1, spsA[:, 1, :])
                nc.scalar.activation(
                    at[:, k0 : k0 + 2, :], spsA, EXP, scale=0.125
                )
                _score_mm(h, qblk, k0 + 2, spsB)
                nc.scalar.activation(at[:, k0 + 2, :], spsB, EXP, scale=0.125)
            _score_mm(h, qblk, 15, spsA[:, 0, :])
            nc.scalar.activation(at[:, 15, :], spsA[:, 0, :], EXP, scale=0.125)

        def ctx_head(cp, h, qblk, buf):
            at = at_sb[buf]
            cps = cp.tile([HD + 1, QB], f32, tag="cps")
            mms = []
            for kt in range(KT):
                for s in range(2):
                    mms.append(nc.tensor.matmul(
                        cps[:, s * 512 : (s + 1) * 512],
                        lhsT=vaug[:, kt, h * (HD + 1) : (h + 1) * (HD + 1)],
                        rhs=at[:, kt, s * 512 : (s + 1) * 512],
                        start=(kt == 0), stop=(kt == KT - 1),
                        skip_group_check=True,
                    ))
            # order the two interleaved accumulation groups
            chain(mms[0::2])
            chain(mms[1::2])
            nc.vector.tensor_copy(
                out=cxt[64 * (h % 2) : 64 * (h % 2) + 64, h // 2,
                        qblk * QB : (qblk + 1) * QB],
                in_=cps[0:HD, :],
            )
            nc.vector.tensor_copy(
                out=scol[0:1, h * QB : (h + 1) * QB], in_=cps[HD : HD + 1, :]
            )

        def normalize(qblk):
            nc.vector.reciprocal(rec, scol)
            nc.sync.dma_start(
                out=screc.rearrange("r q -> (r q)")[
                    qblk * NHEADS * QB : (qblk + 1) * NHEADS * QB
                ],
                in_=rec,
            )
            for kt in range(2):
                bsrc = bass.AP(
                    tensor=screc[:].tensor,
                    offset=(qblk * NHEADS + kt * 2) * QB,
                    ap=[[QB, 2], [0, 64], [1, QB]],
                )
                nc.sync.dma_start(out=rb, in_=bsrc)
                nc.vector.tensor_mul(
                    cxt[:, kt, qblk * QB : (qblk + 1) * QB],
                    cxt[:, kt, qblk * QB : (qblk + 1) * QB],
                    rb,
                )

        sp_cm = tc.tile_pool(name="sps", bufs=1, space="PSUM")
        sp = sp_cm.__enter__()
        spsA = sp.tile([128, 2, QB], f32, tag="spsA")
        spsB = sp.tile([128, QB], f32, tag="spsB")
        qk_cm = tc.tile_pool(name="qkps", bufs=1, space="PSUM")
        qp = qk_cm.__enter__()

        # qk tiles needed by scores(h=0, qblk=0): q01 t-half0, k01 both halves
        qk_tile(qp, 0, 0)
        qk_tile(qp, 2, 0)
        qk_tile(qp, 2, 1)
        scores_exp(sp, 0, 0, 0)
        qk_tile(qp, 0, 1)
        qk_tile(qp, 1, 0)
        qk_tile(qp, 1, 1)
        qk_tile(qp, 3, 0)
        qk_tile(qp, 3, 1)
        qk_cm.__exit__(None, None, None)

        # ---- v projection (natural [t, n]) ----
        v_cm = tc.tile_pool(name="vps", bufs=1, space="PSUM")
        vp = v_cm.__enter__()
        for h in range(NHEADS):
            vps = vp.tile([128, TT * HD], f32, tag="vps")
            mms = []
            for tt in range(TT):
                col = tt * HD
                # bias first so each tt accumulation group is contiguous:
                # PSUM allows only one open group per bank at a time
                mms.append(nc.tensor.matmul(
                    vps[:, col : col + HD],
                    lhsT=ones_sb,
                    rhs=bvrep_sb[0:1, h * 1024 + col : h * 1024 + col + HD],
                    start=True, stop=False, skip_group_check=True,
                ))
                for dt in range(DT):
                    mms.append(nc.tensor.matmul(
                        vps[:, col : col + HD],
                        lhsT=xt_sb[:, dt, tt * 128 : (tt + 1) * 128],
                        rhs=wv_sb[:, dt, h * HD : (h + 1) * HD],
                        start=False, stop=(dt == DT - 1), skip_group_check=True,
                    ))
            chain(mms)
            nc.vector.tensor_copy(
                out=vaug.rearrange("p t (h c) -> p t h c", h=NHEADS)[:, :, h, 0:HD],
                in_=vps.rearrange("p (t c) -> p t c", c=HD),
            )
        v_cm.__exit__(None, None, None)

        # ---- attention (q-block major, ctx lags scores by one group) ----
        cp_cm = tc.tile_pool(name="cps", bufs=1, space="PSUM")
        cp = cp_cm.__enter__()
        groups = [(qb, h) for qb in range(NQB) for h in range(NHEADS)]
        prev = None
        for gi, (qblk, h) in enumerate(groups):
            if not (qblk == 0 and h == 0):
                scores_exp(sp, h, qblk, gi % 2)
            if prev is not None:
                ctx_head(cp, prev[1], prev[0], (gi - 1) % 2)
                if prev[1] == NHEADS - 1:
                    normalize(prev[0])
            prev = (qblk, h)
        ctx_head(cp, prev[1], prev[0], (len(groups) - 1) % 2)
        normalize(prev[0])
        cp_cm.__exit__(None, None, None)
        sp_cm.__exit__(None, None, None)

        if DEBUG_DUMP:
            nc.sync.dma_start(out=dbg["qk"][:, :], in_=qk_sb.rearrange("p m t -> p (m t)"))
            nc.sync.dma_start(out=dbg["vaug"][:, :], in_=vaug.rearrange("p t c -> p (t c)"))
            nc.sync.dma_start(out=dbg["at0"][:, :], in_=at_sb[0].rearrange("p k q -> p (k q)"))
            nc.sync.dma_start(out=dbg["cxt"][:, :], in_=cxt.rearrange("p k t -> p (k t)"))
            nc.sync.dma_start(out=dbg["scol"][:, :], in_=scol)

        # ---- out projection ----
        with (
            tc.tile_pool(name="ops", bufs=2, space="PSUM") as o_p,
            tc.tile_pool(name="ostage", bufs=3) as out_p,
        ):
            for tt in range(TT):
                for nb in range(2):
                    ops = o_p.tile([128, 512], f32, tag="ops")
                    mms = []
                    for kt in range(2):
                        mms.append(nc.tensor.matmul(
                            ops,
                            lhsT=cxt[:, kt, tt * 128 : (tt + 1) * 128],
                            rhs=wout_sb[:, kt, nb * 512 : (nb + 1) * 512],
                            start=(kt == 0), stop=(kt == 1),
                            skip_group_check=True,
                        ))
                    chain(mms)
                    ot = out_p.tile([128, 512], f32, tag="ot")
                    nc.vector.tensor_copy(ot, ops)
                    nc.sync.dma_start(
                        out=out[tt * 128 : (tt + 1) * 128, nb * 512 : (nb + 1) * 512],
                        in_=ot,
                    )

        if loop_cm is not None:
            loop_cm.__exit__(None, None, None)

    return nc


_NC_CACHE = None


def _get_nc():
    global _NC_CACHE
    if _NC_CACHE is None:
        nc = build_nc()
        split_excess_waits(nc)
        _NC_CACHE = nc
    return _NC_CACHE


def make_in_maps(x, Wqkv, bqkv, Wout):
    x = np.asarray(x, dtype=np.float32)
    Wqkv = np.asarray(Wqkv, dtype=np.float32)
    bqkv = np.asarray(bqkv, dtype=np.float32)
    Wout = np.asarray(Wout, dtype=np.float32)
    in_maps = []
    for c in range(NCORES):
        b, g = divmod(c, 4)
        qs = slice(NQK * g, NQK * (g + 1))
        ks = slice(D + NQK * g, D + NQK * (g + 1))
        vs = slice(2 * D + NQK * g, 2 * D + NQK * (g + 1))
        qb, kb, vb = bqkv[qs], bqkv[ks], bqkv[vs]
        bqk_host = np.stack(
            [qb[0:128], qb[128:256], kb[0:128], kb[128:256]], axis=1
        )
        bvrep = np.ascontiguousarray(
            np.broadcast_to(vb.reshape(NHEADS, 1, HD), (NHEADS, 16, HD))
        ).reshape(1, NHEADS * 16 * HD)
        in_maps.append(
            {
                "xt16": np.ascontiguousarray(x[b].T.astype(np.float16)),
                "wqk16": np.ascontiguousarray(
                    np.concatenate([Wqkv[:, qs], Wqkv[:, ks]], axis=1)
                ).astype(np.float16),
                "wv16": np.ascontiguousarray(Wqkv[:, vs]).astype(np.float16),
                "wout16": np.ascontiguousarray(
                    Wout[NQK * g : NQK * (g + 1), :]
                ).astype(np.float16),
                "bqk": np.ascontiguousarray(bqk_host),
                "bvrep": bvrep.astype(np.float16),
            }
        )
    return in_maps


def gather_out(results, bout):
    bout = np.asarray(bout, dtype=np.float32)
    outs = [np.asarray(results[c]["out"], dtype=np.float32) for c in range(NCORES)]
    full = np.stack(
        [outs[4 * b] + outs[4 * b + 1] + outs[4 * b + 2] + outs[4 * b + 3]
         for b in range(B)]
    )
    return (full + bout[None, None, :]).astype(np.float32)


def kernel(x, Wqkv, bqkv, Wout, bout):
    from concourse.bass_utils import run_bass_kernel_spmd

    nc = _get_nc()
    in_maps = make_in_maps(x, Wqkv, bqkv, Wout)
    res = run_bass_kernel_spmd(nc, in_maps, list(range(NCORES)))
    return gather_out(res.results, bout)
